# revision 17
# baseline (speedup 1.0000x reference)
import sys

if "/opt/trn_rl_repo" not in sys.path:
    sys.path.insert(0, "/opt/trn_rl_repo")

import numpy as np

NCORES = 8
B = 65536
NPC = B // NCORES    # 8192 images per core
# pairs of chunk sizes (in 128-image subtiles): quantization runs once per
# pair on GPSIMD (its ~4us fixed dispatch cost only amortizes on big ops);
# T/conv/pool/FC run per half-pair
PAIRS = [(1, 3)] + [(4, 4)] * 7 + [(3, 1)]
assert sum(a + b for a, b in PAIRS) == NPC // 128
AF = 128.0 / 127.5

# conv as 5 aligned banded matmuls: input-pixel block k covers pixels
# [128k, 128k+128) (block 4 holds only the 64 real pixels 512..575).
# Output pixels are split into segments; a segment (k, lo, hi, start,
# stop) accumulates block k's contribution to outputs [lo, hi].  PAB
# lives at PSUM cols [448, 1024) so no segment crosses the 2KB bank edge
# (the edge falls at o=64).
SEGS = [
    (0, 0, 63, True, True),
    (0, 64, 152, True, False),
    (1, 103, 152, False, True),
    (1, 153, 280, True, False),
    (2, 231, 280, False, True),
    (2, 281, 408, True, False),
    (3, 359, 408, False, True),
    (3, 409, 536, True, False),
    (4, 487, 536, False, True),
    (4, 537, 575, True, True),
]
POFF = 448  # PAB column offset

# wc column offset of each (k, lo) segment, packed in SEGS order
_WCOFF = {}
_off = 0
for (_k, _lo, _hi, _st, _sp) in SEGS:
    _WCOFF[(_k, _lo)] = _off
    _off += _hi - _lo + 1
assert _off == 776

_cache = {}


def _build():
    from contextlib import ExitStack

    import concourse.tile as tile
    from concourse import bacc, mybir

    f32 = mybir.dt.float32
    f16 = mybir.dt.float16
    Alu = mybir.AluOpType
    Act = mybir.ActivationFunctionType

    nc = bacc.Bacc("TRN2", target_bir_lowering=False, debug=False,
                   num_devices=NCORES)
    x = nc.dram_tensor("x", [NPC, 576], f32, kind="ExternalInput").ap()
    wcv = nc.dram_tensor("wcv", [128, 776], f16, kind="ExternalInput").ap()
    wfc = nc.dram_tensor("wfc", [256, 10], f16, kind="ExternalInput").ap()
    ident = nc.dram_tensor("ident", [128, 128], f16, kind="ExternalInput").ap()
    out = nc.dram_tensor("out", [10, NPC], f32, kind="ExternalOutput").ap()

    with tile.TileContext(nc) as tc, ExitStack() as ctx:
        consts = ctx.enter_context(tc.tile_pool(name="consts", bufs=1))
        wc = consts.tile([128, 776], f16)
        idt = consts.tile([128, 128], f16)
        w1 = consts.tile([128, 10], f16)
        w2 = consts.tile([128, 10], f16)
        nbias = consts.tile([128, 1], f32)
        nc.gpsimd.memset(nbias[:], -1536.0)
        nc.sync.dma_start(wc[:], wcv)
        nc.sync.dma_start(idt[:], ident)
        nc.sync.dma_start(w1[:], wfc[0:128, :])
        nc.sync.dma_start(w2[:], wfc[128:256, :])

        xpool = ctx.enter_context(tc.tile_pool(name="xp", bufs=2))
        qpool = ctx.enter_context(tc.tile_pool(name="qp", bufs=2))
        xtpool = ctx.enter_context(tc.tile_pool(name="xtp", bufs=3))
        actpool = ctx.enter_context(tc.tile_pool(name="actp", bufs=2))
        atpool = ctx.enter_context(tc.tile_pool(name="atp", bufs=2))
        sopool = ctx.enter_context(tc.tile_pool(name="sop", bufs=2))
        pst = ctx.enter_context(tc.tile_pool(name="pst", bufs=2, space="PSUM"))
        pca = ctx.enter_context(tc.tile_pool(name="pca", bufs=2, space="PSUM"))
        pfc = ctx.enter_context(tc.tile_pool(name="pfc", bufs=2, space="PSUM"))

        xv_dram = x.rearrange("(t p) f -> p t f", p=128)

        # flatten pair structure into half-chunks with their pair-level
        # quantized slice; quantization runs once per pair on GPSIMD (its
        # fixed cost only amortizes on big ops)
        halves = []  # (q_ap, n, sb)
        sb = 0
        for (n0, n1) in PAIRS:
            m = n0 + n1
            xr = xpool.tile([128, m * 576], f32, tag="xr")
            nc.sync.dma_start(xr[:].rearrange("p (a f) -> p a f", a=m),
                              xv_dram[:, sb:sb + m, :])
            # q = RTNE_f16(x*AF + (1536-128)) -- exact int in [1408, 1664]
            qp = qpool.tile([128, m * 576], f16, tag="q")
            nc.gpsimd.tensor_scalar(qp[:], xr[:], AF, 1536.0 - 128.0,
                                    Alu.mult, Alu.add)
            halves.append((qp[:, 0:n0 * 576], n0, sb))
            halves.append((qp[:, n0 * 576:m * 576], n1, sb + n0))
            sb += m

        def stage_T(q, n):
            # transpose to pixel-major via the PE transpose path -- writes
            # f16 PSUM (16-bit evacuation reads run in DVE 2x mode)
            xt = xtpool.tile([128, 5 * n * 128], f16, tag="xt")
            for k in range(5):
                pk = 64 if k == 4 else 128
                T = pst.tile([128, n * 128], f16, tag="ps", name=f"T{k}")
                for a in range(n):
                    nc.tensor.transpose(
                        T[0:pk, a * 128:(a + 1) * 128],
                        q[:, a * 576 + 128 * k:a * 576 + 128 * k + pk],
                        idt[:])
                xtk = xt[0:pk, k * n * 128:(k + 1) * n * 128]
                if k == 4:
                    nc.vector.tensor_scalar(xtk, T[0:pk, :], 1536.0,
                                            127.0, Alu.subtract, Alu.min)
                else:
                    nc.scalar.activation(xtk, T[0:pk, :], Act.Copy,
                                         bias=-1536.0)
            return xt

        def stage_conv(xt, n):
            # conv: per subtile, 10 banded matmuls with PSUM accumulation
            # on block-boundary ranges, then the full 2x2 maxpool straight
            # out of conv PSUM in one tensor_reduce (f16 write exact:
            # conv outputs are half-integers <= 576)
            act = actpool.tile([128, n * 144], f16, tag="act")
            for a in range(n):
                PAB = pca.tile([128, 1024], f32)
                for (k, lo, hi, st, sp) in SEGS:
                    pk = 64 if k == 4 else 128
                    lhs = xt[0:pk, (k * n + a) * 128:(k * n + a + 1) * 128]
                    nc.tensor.matmul(
                        PAB[:, POFF + lo:POFF + hi + 1], lhs,
                        wc[0:pk, _WCOFF[(k, lo)]:
                           _WCOFF[(k, lo)] + hi - lo + 1],
                        start=st, stop=sp, skip_group_check=True)
                pv = PAB[:, POFF:POFF + 576].rearrange(
                    "p (r tr c tc) -> p r c tr tc", r=12, tr=2, c=12)
                o1 = act[:, a * 144:(a + 1) * 144].rearrange(
                    "p (r c) -> p r c", r=12)
                nc.vector.tensor_reduce(o1, pv, mybir.AxisListType.XY,
                                        Alu.max)
            # +1536, clip at 1663=127+1536; f16 write rounds to int.  act
            # stays biased; negatives stay below 1536 -- the aT
            # evacuations' fused relu applies the low clip.
            act2 = actpool.tile([128, n * 144], f16, tag="act2")
            nc.gpsimd.tensor_scalar(act2[:], act[:], 1536.0, 1663.0,
                                    Alu.add, Alu.min)
            # transpose act to feature-major (f16 PSUM again)
            pT1 = pfc.tile([128, n * 128], f16, tag="fc")
            pT2 = pfc.tile([128, n * 128], f16, tag="fc")
            for a in range(n):
                nc.tensor.transpose(pT1[:, a * 128:(a + 1) * 128],
                                    act2[:, a * 144:a * 144 + 128], idt[:])
                nc.tensor.transpose(pT2[:, a * 128:(a + 1) * 128],
                                    act2[:, a * 144 + 16:a * 144 + 144],
                                    idt[:])
            # aT = relu(pT - 1536)
            aT1 = atpool.tile([128, n * 128], f16, tag="aT1")
            aT2 = atpool.tile([128, n * 128], f16, tag="aT2")
            nc.vector.tensor_scalar(aT1[:], pT1[:], 1536.0, 0.0,
                                    Alu.subtract, Alu.max)
            nc.scalar.activation(aT2[:], pT2[:], Act.Relu, bias=nbias[:])
            return aT1, aT2

        def stage_fc(aT1, aT2, n, sbh):
            pOT = pfc.tile([10, n * 128], f32, tag="fc")
            nc.tensor.matmul(pOT[:], w1[:], aT1[:], start=True, stop=False)
            nc.tensor.matmul(pOT[:], w2[:], aT2[:], start=False, stop=True)
            so = sopool.tile([10, n * 128], f32, tag="so")
            nc.scalar.copy(so[:], pOT[:])
            nc.sync.dma_start(out[:, sbh * 128:sbh * 128 + n * 128], so[:])

        # lag-2 pipeline: PE order per step is [T(h)], [conv(h-1)],
        # [poolT(h-1)], [FC(h-2)] so the PE is never waiting on an
        # evacuation it just scheduled
        tq = []  # (xt, n, sb) awaiting conv
        cq = []  # (aT1, aT2, n, sb) awaiting fc
        for (q, n, sbh) in halves:
            xt = stage_T(q, n)
            if tq:
                (xt_p, n_p, sb_p) = tq.pop(0)
                cq.append(stage_conv(xt_p, n_p) + (n_p, sb_p))
            if len(cq) >= 2:
                stage_fc(*cq.pop(0))
            tq.append((xt, n, sbh))
        while tq:
            (xt_p, n_p, sb_p) = tq.pop(0)
            cq.append(stage_conv(xt_p, n_p) + (n_p, sb_p))
        while cq:
            stage_fc(*cq.pop(0))

    nc.compile()
    return nc


def _prep(conv_w, fc_w):
    cw = np.asarray(conv_w, np.float32).reshape(3, 3)
    wq = (np.round(np.clip(cw, -0.5, 0.5) * 2.0) / 2.0).astype(np.float32)
    fw = np.asarray(fc_w, np.float32)
    wfq = (np.round(np.clip(fw, -0.5, 0.5) * 2.0) / 2.0 / 8.0).astype(np.float32)

    # banded conv matrix W[in_pix, out_pix]
    W = np.zeros((640, 576), np.float32)
    for r in range(24):
        for c in range(24):
            o = r * 24 + c
            for dr in (-1, 0, 1):
                for dc in (-1, 0, 1):
                    rr, cc = r + dr, c + dc
                    if 0 <= rr < 24 and 0 <= cc < 24:
                        W[rr * 24 + cc, o] += wq[dr + 1, dc + 1]
    wcv = np.zeros((128, 776), np.float32)
    for (k, lo, hi, st, sp) in SEGS:
        off = _WCOFF[(k, lo)]
        wcv[:, off:off + hi - lo + 1] = W[128 * k:128 * k + 128, lo:hi + 1]

    Wdev = np.zeros((256, 10), np.float32)
    for i in range(12):
        for j in range(12):
            kk = i * 12 + j
            r = kk if kk < 128 else kk + 112
            Wdev[r, :] = wfq[:, (i + 1) * 14 + (j + 1)] / 128.0
    identm = np.eye(128, dtype=np.float16)
    return (wcv.astype(np.float16), Wdev.astype(np.float16), identm)


def _get_program():
    nc = _cache.get("prog")
    if nc is None:
        nc = _build()
        _cache["prog"] = nc
    return nc


def run(x, conv_w, fc_w, trace=False, **kw):
    from concourse.bass_utils import run_bass_kernel_spmd

    x2d = np.ascontiguousarray(np.asarray(x, np.float32).reshape(B, 576))
    wcv, Wdev, identm = _prep(conv_w, fc_w)
    nc = _get_program()
    in_maps = [{"x": np.ascontiguousarray(x2d[c * NPC:(c + 1) * NPC]),
                "wcv": wcv, "wfc": Wdev, "ident": identm}
               for c in range(NCORES)]
    res = run_bass_kernel_spmd(nc, in_maps,
                               core_ids=list(range(NCORES)),
                               trace=trace, **kw)
    out = np.concatenate([np.asarray(r["out"]).T for r in res.results], axis=0)
    return np.ascontiguousarray(out.astype(np.float32)), res


def kernel(x, conv_w, fc_w):
    out, _ = run(x, conv_w, fc_w, trace=False)
    return out


# revision 18
# speedup vs baseline: 1.7237x; 1.7237x over previous
import sys

if "/opt/trn_rl_repo" not in sys.path:
    sys.path.insert(0, "/opt/trn_rl_repo")

import numpy as np

NCORES = 8
B = 65536
NPC = B // NCORES    # 8192 images per core
# pairs of chunk sizes (in 128-image subtiles): quantization runs once per
# pair on GPSIMD (its ~4us fixed dispatch cost only amortizes on big ops);
# T/conv/pool/FC run per half-pair
PAIRS = [(1, 3)] + [(4, 4)] * 7 + [(3, 1)]
assert sum(a + b for a, b in PAIRS) == NPC // 128
AF = 128.0 / 127.5

# conv as 5 aligned banded matmuls: input-pixel block k covers pixels
# [128k, 128k+128) (block 4 holds only the 64 real pixels 512..575).
# Output pixels are split into segments; a segment (k, lo, hi, start,
# stop) accumulates block k's contribution to outputs [lo, hi].  PAB
# lives at PSUM cols [448, 1024) so no segment crosses the 2KB bank edge
# (the edge falls at o=64).
SEGS = [
    (0, 0, 63, True, True),
    (0, 64, 152, True, False),
    (1, 103, 152, False, True),
    (1, 153, 280, True, False),
    (2, 231, 280, False, True),
    (2, 281, 408, True, False),
    (3, 359, 408, False, True),
    (3, 409, 536, True, False),
    (4, 487, 536, False, True),
    (4, 537, 575, True, True),
]
POFF = 448  # PAB column offset

# wc column offset of each (k, lo) segment, packed in SEGS order
_WCOFF = {}
_off = 0
for (_k, _lo, _hi, _st, _sp) in SEGS:
    _WCOFF[(_k, _lo)] = _off
    _off += _hi - _lo + 1
assert _off == 776

_cache = {}


def _build():
    from contextlib import ExitStack

    import concourse.tile as tile
    from concourse import bacc, mybir

    f32 = mybir.dt.float32
    f16 = mybir.dt.float16
    Alu = mybir.AluOpType
    Act = mybir.ActivationFunctionType

    nc = bacc.Bacc("TRN2", target_bir_lowering=False, debug=False,
                   num_devices=NCORES)
    x = nc.dram_tensor("x", [NPC, 576], f32, kind="ExternalInput").ap()
    wcv = nc.dram_tensor("wcv", [128, 776], f16, kind="ExternalInput").ap()
    wfc = nc.dram_tensor("wfc", [256, 10], f16, kind="ExternalInput").ap()
    ident = nc.dram_tensor("ident", [128, 128], f16, kind="ExternalInput").ap()
    out = nc.dram_tensor("out", [10, NPC], f32, kind="ExternalOutput").ap()

    with tile.TileContext(nc) as tc, ExitStack() as ctx:
        consts = ctx.enter_context(tc.tile_pool(name="consts", bufs=1))
        wc = consts.tile([128, 776], f16)
        idt = consts.tile([128, 128], f16)
        w1 = consts.tile([128, 10], f16)
        w2 = consts.tile([128, 10], f16)
        nbias = consts.tile([128, 1], f32)
        nc.gpsimd.memset(nbias[:], -1536.0)
        nc.sync.dma_start(wc[:], wcv)
        nc.sync.dma_start(idt[:], ident)
        nc.sync.dma_start(w1[:], wfc[0:128, :])
        nc.sync.dma_start(w2[:], wfc[128:256, :])

        xpool = ctx.enter_context(tc.tile_pool(name="xp", bufs=2))
        qpool = ctx.enter_context(tc.tile_pool(name="qp", bufs=2))
        xtpool = ctx.enter_context(tc.tile_pool(name="xtp", bufs=3))
        actpool = ctx.enter_context(tc.tile_pool(name="actp", bufs=2))
        atpool = ctx.enter_context(tc.tile_pool(name="atp", bufs=2))
        sopool = ctx.enter_context(tc.tile_pool(name="sop", bufs=2))
        pst = ctx.enter_context(tc.tile_pool(name="pst", bufs=2, space="PSUM"))
        pca = ctx.enter_context(tc.tile_pool(name="pca", bufs=2, space="PSUM"))
        pfc = ctx.enter_context(tc.tile_pool(name="pfc", bufs=2, space="PSUM"))

        xv_dram = x.rearrange("(t p) f -> p t f", p=128)

        # flatten pair structure into half-chunks with their pair-level
        # quantized slice; quantization runs once per pair on GPSIMD (its
        # fixed cost only amortizes on big ops)
        halves = []  # (q_ap, n, sb)
        sb = 0
        for (n0, n1) in PAIRS:
            m = n0 + n1
            xr = xpool.tile([128, m * 576], f32, tag="xr")
            nc.sync.dma_start(xr[:].rearrange("p (a f) -> p a f", a=m),
                              xv_dram[:, sb:sb + m, :])
            # q = RTNE_f16(x*AF + (1536-128)) -- exact int in [1408, 1664]
            qp = qpool.tile([128, m * 576], f16, tag="q")
            nc.gpsimd.tensor_scalar(qp[:], xr[:], AF, 1536.0 - 128.0,
                                    Alu.mult, Alu.add)
            halves.append((qp[:, 0:n0 * 576], n0, sb))
            halves.append((qp[:, n0 * 576:m * 576], n1, sb + n0))
            sb += m

        def stage_T(q, n):
            # transpose to pixel-major via the PE transpose path -- writes
            # f16 PSUM (16-bit evacuation reads run in DVE 2x mode)
            xt = xtpool.tile([128, 5 * n * 128], f16, tag="xt")
            for k in range(5):
                pk = 64 if k == 4 else 128
                T = pst.tile([128, n * 128], f16, tag="ps", name=f"T{k}")
                for a in range(n):
                    nc.tensor.transpose(
                        T[0:pk, a * 128:(a + 1) * 128],
                        q[:, a * 576 + 128 * k:a * 576 + 128 * k + pk],
                        idt[:])
                xtk = xt[0:pk, k * n * 128:(k + 1) * n * 128]
                if k == 4:
                    nc.vector.tensor_scalar(xtk, T[0:pk, :], 1536.0,
                                            127.0, Alu.subtract, Alu.min)
                else:
                    nc.scalar.activation(xtk, T[0:pk, :], Act.Copy,
                                         bias=-1536.0)
            return xt

        def stage_conv(xt, n):
            # conv: per subtile, 10 banded matmuls with PSUM accumulation
            # on block-boundary ranges, then the full 2x2 maxpool straight
            # out of conv PSUM in one tensor_reduce (f16 write exact:
            # conv outputs are half-integers <= 576)
            act = actpool.tile([128, n * 144], f16, tag="act")
            for a in range(n):
                PAB = pca.tile([128, 1024], f32)
                for (k, lo, hi, st, sp) in SEGS:
                    pk = 64 if k == 4 else 128
                    lhs = xt[0:pk, (k * n + a) * 128:(k * n + a + 1) * 128]
                    nc.tensor.matmul(
                        PAB[:, POFF + lo:POFF + hi + 1], lhs,
                        wc[0:pk, _WCOFF[(k, lo)]:
                           _WCOFF[(k, lo)] + hi - lo + 1],
                        start=st, stop=sp, skip_group_check=True)
                pv = PAB[:, POFF:POFF + 576].rearrange(
                    "p (r tr c tc) -> p r c tr tc", r=12, tr=2, c=12)
                o1 = act[:, a * 144:(a + 1) * 144].rearrange(
                    "p (r c) -> p r c", r=12)
                nc.vector.tensor_reduce(o1, pv, mybir.AxisListType.XY,
                                        Alu.max)
            # +1536, clip at 1663=127+1536; f16 write rounds to int.  act
            # stays biased; negatives stay below 1536 -- the aT
            # evacuations' fused relu applies the low clip.
            act2 = act
            nc.vector.tensor_scalar(act2[:], act[:], 1536.0, 1663.0,
                                    Alu.add, Alu.min)
            # transpose act to feature-major (f16 PSUM again)
            pT1 = pfc.tile([128, n * 128], f16, tag="fc")
            pT2 = pfc.tile([128, n * 128], f16, tag="fc")
            for a in range(n):
                nc.tensor.transpose(pT1[:, a * 128:(a + 1) * 128],
                                    act2[:, a * 144:a * 144 + 128], idt[:])
                nc.tensor.transpose(pT2[:, a * 128:(a + 1) * 128],
                                    act2[:, a * 144 + 16:a * 144 + 144],
                                    idt[:])
            # aT = relu(pT - 1536)
            aT1 = atpool.tile([128, n * 128], f16, tag="aT1")
            aT2 = atpool.tile([128, n * 128], f16, tag="aT2")
            nc.vector.tensor_scalar(aT1[:], pT1[:], 1536.0, 0.0,
                                    Alu.subtract, Alu.max)
            nc.scalar.activation(aT2[:], pT2[:], Act.Relu, bias=nbias[:])
            return aT1, aT2

        def stage_fc(aT1, aT2, n, sbh):
            pOT = pfc.tile([10, n * 128], f32, tag="fc")
            nc.tensor.matmul(pOT[:], w1[:], aT1[:], start=True, stop=False)
            nc.tensor.matmul(pOT[:], w2[:], aT2[:], start=False, stop=True)
            so = sopool.tile([10, n * 128], f32, tag="so")
            nc.scalar.copy(so[:], pOT[:])
            nc.sync.dma_start(out[:, sbh * 128:sbh * 128 + n * 128], so[:])

        # lag-2 pipeline: PE order per step is [T(h)], [conv(h-1)],
        # [poolT(h-1)], [FC(h-2)] so the PE is never waiting on an
        # evacuation it just scheduled
        tq = []  # (xt, n, sb) awaiting conv
        cq = []  # (aT1, aT2, n, sb) awaiting fc
        for (q, n, sbh) in halves:
            xt = stage_T(q, n)
            if tq:
                (xt_p, n_p, sb_p) = tq.pop(0)
                cq.append(stage_conv(xt_p, n_p) + (n_p, sb_p))
            if len(cq) >= 2:
                stage_fc(*cq.pop(0))
            tq.append((xt, n, sbh))
        while tq:
            (xt_p, n_p, sb_p) = tq.pop(0)
            cq.append(stage_conv(xt_p, n_p) + (n_p, sb_p))
        while cq:
            stage_fc(*cq.pop(0))

    nc.compile()
    return nc


def _prep(conv_w, fc_w):
    cw = np.asarray(conv_w, np.float32).reshape(3, 3)
    wq = (np.round(np.clip(cw, -0.5, 0.5) * 2.0) / 2.0).astype(np.float32)
    fw = np.asarray(fc_w, np.float32)
    wfq = (np.round(np.clip(fw, -0.5, 0.5) * 2.0) / 2.0 / 8.0).astype(np.float32)

    # banded conv matrix W[in_pix, out_pix]
    W = np.zeros((640, 576), np.float32)
    for r in range(24):
        for c in range(24):
            o = r * 24 + c
            for dr in (-1, 0, 1):
                for dc in (-1, 0, 1):
                    rr, cc = r + dr, c + dc
                    if 0 <= rr < 24 and 0 <= cc < 24:
                        W[rr * 24 + cc, o] += wq[dr + 1, dc + 1]
    wcv = np.zeros((128, 776), np.float32)
    for (k, lo, hi, st, sp) in SEGS:
        off = _WCOFF[(k, lo)]
        wcv[:, off:off + hi - lo + 1] = W[128 * k:128 * k + 128, lo:hi + 1]

    Wdev = np.zeros((256, 10), np.float32)
    for i in range(12):
        for j in range(12):
            kk = i * 12 + j
            r = kk if kk < 128 else kk + 112
            Wdev[r, :] = wfq[:, (i + 1) * 14 + (j + 1)] / 128.0
    identm = np.eye(128, dtype=np.float16)
    return (wcv.astype(np.float16), Wdev.astype(np.float16), identm)


def _get_program():
    nc = _cache.get("prog")
    if nc is None:
        nc = _build()
        _cache["prog"] = nc
    return nc


def run(x, conv_w, fc_w, trace=False, **kw):
    from concourse.bass_utils import run_bass_kernel_spmd

    x2d = np.ascontiguousarray(np.asarray(x, np.float32).reshape(B, 576))
    wcv, Wdev, identm = _prep(conv_w, fc_w)
    nc = _get_program()
    in_maps = [{"x": np.ascontiguousarray(x2d[c * NPC:(c + 1) * NPC]),
                "wcv": wcv, "wfc": Wdev, "ident": identm}
               for c in range(NCORES)]
    res = run_bass_kernel_spmd(nc, in_maps,
                               core_ids=list(range(NCORES)),
                               trace=trace, **kw)
    out = np.concatenate([np.asarray(r["out"]).T for r in res.results], axis=0)
    return np.ascontiguousarray(out.astype(np.float32)), res


def kernel(x, conv_w, fc_w):
    out, _ = run(x, conv_w, fc_w, trace=False)
    return out
